# revision 9
# baseline (speedup 1.0000x reference)
"""Trainium2 Bass kernel for AttentionPoolCompressor.

Computation (matches the reference nn.Module):
    x = chunk.reshape(N, 4, 512)
    scores = einsum('d,nrd->nr', query, x) / sqrt(512)
    attn   = softmax(scores, axis=-1)
    pooled = einsum('nr,nrd->nd', attn, x)
    out    = pooled @ w.T + b

Sharding: chunk rows are split contiguously across 8 NeuronCores (each pools
its own L/8 rows independently); query / w / b are replicated.  No
collectives; each core writes its own slice of the output.

Design (v5).  HBM floor: 64MiB fp32 in + 8MiB bf16 out + consts ~= 76.7MB
@358GB/s = 214us; the kernel aims to keep the DMA stream gap-free and the
compute cadence strictly below the 13.2us/tile DMA cadence.
  * Variable tile schedule: 4x512-row staged sub-tiles (HWDGE fp32 +
    DVE cast; compute starts ~8us), 14x2048-row SWDGE-cast main tiles,
    4x512-row SWDGE tail sub-tiles (shrinks the post-last-input drain
    chain from ~22us to ~8us).
  * Consts (q, id, wt, ones, b) are FIRST in the HWDGE FIFO — in v4 they
    sat behind 8.4MB of stage loads and gated the first diag/proj at 55us.
  * xp bufs=6: with 5, tile k+5's SWDGE load waited on PE(k) retiring its
    x tile, starving the input stream for ~20us mid-kernel.
  * Output is bf16 (halves output HBM bytes; host upcasts — measured
    rel err 4.9e-3 vs the 2e-2 gate).
  * Scores: bf16 2x-mode pair-add tree (D 512->256->128->64) + segmented
    tensor_reduce.  Fused accum ops (tensor_tensor_reduce ISA op crashes
    the exec unit; scalar_tensor_tensor / custom affine_mul_reduce run
    1x-mode ~0.77us/j) all lose to the tree (~9.7us/tile).
  * Engine split per 2048-row tile (measured: DVE TT bf16 0.55ns/e 2x,
    reduce 1.08, ACT 0.54ns/e + 410ns overhead, PE ~0.56ns/beat):
      DVE ~12.0us: cast (staged only), score tree, group sums+recip,
        12/16 of the diag build (broadcast TT, 1x).
      ACT ~11.7us: exp (bf16 out), 4/16 diag, pooled copy (folds 1/sum
        as per-partition scale), pooledT copy, out copy.
      PE  ~10.7us: pool (diag matmuls), transposes, proj, bias matmul.
  * Two-tile-lag software pipeline as before (scores i | softmax i-1 |
    PE i-2) so no in-order engine queue head-blocks.
"""

import math
import sys

import numpy as np

if "/opt/trn_rl_repo" not in sys.path:
    sys.path.insert(0, "/opt/trn_rl_repo")

D = 512
RATIO = 4
N_CORES = 8
L_FULL = 262144
ROWS_PER_CORE = L_FULL // N_CORES  # 32768

MAIN_ROWS = 2048  # main tile size
SUB_ROWS = 512  # staged/tail sub-tile size
N_STAGED = 4  # leading sub-tiles via HWDGE fp32 + DVE cast
N_TAIL = 4  # trailing sub-tiles (short drain chain)

OUT_BF16 = True

_NC_CACHE = {}


def _tile_list(rows):
    """[(row_start, nrows, staged)] — staged prefix, main body, small tail."""
    tiles = [(i * SUB_ROWS, SUB_ROWS, True) for i in range(N_STAGED)]
    r = N_STAGED * SUB_ROWS
    tail = N_TAIL * SUB_ROWS
    while rows - r - tail >= MAIN_ROWS:
        tiles.append((r, MAIN_ROWS, False))
        r += MAIN_ROWS
    while r < rows:
        tiles.append((r, SUB_ROWS, False))
        r += SUB_ROWS
    assert r == rows
    return tiles


def _build_nc(rows_per_core, reps=1):
    import contextlib
    from contextlib import ExitStack

    import concourse.bacc as bacc
    import concourse.tile as tile
    from concourse import mybir

    fp32 = mybir.dt.float32
    bf16 = mybir.dt.bfloat16
    Alu = mybir.AluOpType
    Act = mybir.ActivationFunctionType
    X = mybir.AxisListType.X

    tiles = _tile_list(rows_per_core)
    n_tiles = len(tiles)
    out_rows = rows_per_core // RATIO
    inv_sqrt_d = 1.0 / math.sqrt(D)

    nc = bacc.Bacc("TRN2", target_bir_lowering=False, debug=False)
    chunk = nc.dram_tensor("chunk", [rows_per_core, D], fp32, kind="ExternalInput").ap()
    wtb = nc.dram_tensor("wtb", [D, D], bf16, kind="ExternalInput").ap()
    qbc = nc.dram_tensor("qbc", [128, D], bf16, kind="ExternalInput").ap()
    ident = nc.dram_tensor("ident", [128, 128], bf16, kind="ExternalInput").ap()
    ones1 = nc.dram_tensor("ones1", [1, 128], bf16, kind="ExternalInput").ap()
    brow = nc.dram_tensor("brow", [1, D], bf16, kind="ExternalInput").ap()
    out_dt = bf16 if OUT_BF16 else fp32
    out = nc.dram_tensor("out", [out_rows, D], out_dt, kind="ExternalOutput").ap()

    JMAX = MAIN_ROWS // 128  # 16

    with tile.TileContext(nc) as tc, ExitStack() as ctx:
        const = ctx.enter_context(tc.tile_pool(name="const", bufs=1))
        xp = ctx.enter_context(tc.tile_pool(name="xp", bufs=6))
        xps = ctx.enter_context(tc.tile_pool(name="xps", bufs=5))
        prp = ctx.enter_context(tc.tile_pool(name="prp", bufs=1))
        l1p = ctx.enter_context(tc.tile_pool(name="l1p", bufs=1))
        l2p = ctx.enter_context(tc.tile_pool(name="l2p", bufs=1))
        l3p = ctx.enter_context(tc.tile_pool(name="l3p", bufs=1))
        s_p = ctx.enter_context(tc.tile_pool(name="s_p", bufs=3))
        e_p = ctx.enter_context(tc.tile_pool(name="e_p", bufs=3))
        gs_p = ctx.enter_context(tc.tile_pool(name="gs_p", bufs=3))
        rec_p = ctx.enter_context(tc.tile_pool(name="rec_p", bufs=3))
        dp = ctx.enter_context(tc.tile_pool(name="dp", bufs=3))
        pooledp = ctx.enter_context(tc.tile_pool(name="pooledp", bufs=2))
        ptp = ctx.enter_context(tc.tile_pool(name="ptp", bufs=2))
        outp = ctx.enter_context(tc.tile_pool(name="outp", bufs=3))
        stagep = ctx.enter_context(tc.tile_pool(name="stagep", bufs=3))
        ps_pool = ctx.enter_context(tc.tile_pool(name="ps_pool", bufs=2, space="PSUM"))
        ps_pt = ctx.enter_context(tc.tile_pool(name="ps_pt", bufs=2, space="PSUM"))
        ps_o = ctx.enter_context(tc.tile_pool(name="ps_o", bufs=4, space="PSUM"))

        wt_t = const.tile([128, 4 * D], bf16)
        q_t = const.tile([128, D], bf16)
        id_t = const.tile([128, 128], bf16)
        ones_t = const.tile([1, 128], bf16)
        b_t = const.tile([1, D], bf16)

        def load_consts():
            # consts go FIRST in the HWDGE FIFO: q for scores, id for
            # diag/transposes, wt/ones/b for the proj — all needed within
            # the first ~10us.
            nc.sync.dma_start(out=q_t[:], in_=qbc[:, :])
            nc.sync.dma_start(out=id_t[:], in_=ident[:, :])
            nc.sync.dma_start(out=ones_t[:], in_=ones1[:, :])
            nc.sync.dma_start(out=b_t[:], in_=brow[:, :])
            for c in range(4):
                nc.sync.dma_start(
                    out=wt_t[:, c * D : (c + 1) * D],
                    in_=wtb[c * 128 : (c + 1) * 128, :],
                )

        def src_ap(start, nrows):
            j = nrows // 128
            return chunk[start : start + nrows, :].rearrange("(p j) d -> p (j d)", j=j)

        def load_tile(start, nrows):
            j = nrows // 128
            pool = xp if nrows == MAIN_ROWS else xps
            x_t = pool.tile([128, j * D], bf16)
            nc.gpsimd.dma_start(out=x_t[:], in_=src_ap(start, nrows))
            return x_t

        def stage_dma(start, nrows):
            j = nrows // 128
            x_t = xps.tile([128, j * D], bf16)
            stage = stagep.tile([128, j * D], fp32)
            nc.sync.dma_start(out=stage[:], in_=src_ap(start, nrows))
            return x_t, stage

        def cast_stage(x_t, stage):
            nc.vector.tensor_copy(x_t[:], stage[:])

        def scores_front(st):
            """DVE: 2x-mode pair-add tree + segmented reduce -> s [128,J]."""
            x_t, J = st["x"], st["J"]
            pv = x_t[:].rearrange("p (j d) -> p j d", j=J)
            pr = prp.tile([128, J * D], bf16)
            prv = pr[:].rearrange("p (j d) -> p j d", j=J)
            nc.vector.tensor_tensor(
                prv, pv, q_t[:].unsqueeze(1).broadcast_to((128, J, D)), Alu.mult
            )
            l1 = l1p.tile([128, J * 256], bf16)
            v1 = l1[:].rearrange("p (j d) -> p j d", j=J)
            nc.vector.tensor_tensor(v1, prv[:, :, 0:256], prv[:, :, 256:512], Alu.add)
            l2 = l2p.tile([128, J * 128], bf16)
            v2 = l2[:].rearrange("p (j d) -> p j d", j=J)
            nc.vector.tensor_tensor(v2, v1[:, :, 0:128], v1[:, :, 128:256], Alu.add)
            l3 = l3p.tile([128, J * 64], bf16)
            v3 = l3[:].rearrange("p (j d) -> p j d", j=J)
            nc.vector.tensor_tensor(v3, v2[:, :, 0:64], v2[:, :, 64:128], Alu.add)
            s_t = s_p.tile([128, J], fp32)
            nc.vector.tensor_reduce(s_t[:], v3, axis=X, op=Alu.add)
            st["s"] = s_t

        def exp_stage(st):
            J = st["J"]
            e_t = e_p.tile([128, J], fp32)  # ACT diag scale AP must be fp32
            nc.scalar.activation(
                out=e_t[:], in_=st["s"][:], func=Act.Exp, scale=inv_sqrt_d
            )
            st["e"] = e_t

        def softmax_finish(st):
            J, G = st["J"], st["J"] // RATIO
            gs_t = gs_p.tile([128, G], fp32)
            nc.vector.tensor_reduce(
                gs_t[:],
                st["e"][:].rearrange("p (g r) -> p g r", g=G),
                axis=X,
                op=Alu.add,
            )
            rec_t = rec_p.tile([128, G], fp32)
            nc.vector.reciprocal(rec_t[:], gs_t[:])
            st["rec"] = rec_t

        def d_stage(st):
            """UNNORMALIZED diag(e) blocks; 3/4 on DVE, 1/4 on ACT."""
            J = st["J"]
            dve_j = (3 * J) // 4
            d_t = dp.tile([128, J * 128], bf16)
            nc.vector.tensor_tensor(
                d_t[:, : dve_j * 128].rearrange("p (j m) -> p j m", j=dve_j),
                id_t[:].unsqueeze(1).broadcast_to((128, dve_j, 128)),
                st["e"][:, :dve_j].unsqueeze(2).broadcast_to((128, dve_j, 128)),
                Alu.mult,
            )
            for j in range(dve_j, J):
                nc.scalar.activation(
                    out=d_t[:, j * 128 : (j + 1) * 128],
                    in_=id_t[:],
                    func=Act.Copy,
                    scale=st["e"][:, j : j + 1],
                )
            st["d"] = d_t

        def pe_block(st):
            x_t, d_t, rec_t = st["x"], st["d"], st["rec"]
            start_row, J = st["start"], st["J"]
            G = J // RATIO
            out_sb = outp.tile([128, G * D], bf16)
            for g in range(G):
                pool_ps = ps_pool.tile([128, D], fp32)
                for r in range(RATIO):
                    j = g * RATIO + r
                    nc.tensor.matmul(
                        out=pool_ps[:],
                        lhsT=d_t[:, j * 128 : (j + 1) * 128],
                        rhs=x_t[:, j * D : (j + 1) * D],
                        start=(r == 0),
                        stop=(r == RATIO - 1),
                    )
                pooled_sb = pooledp.tile([128, D], bf16)
                nc.scalar.activation(
                    out=pooled_sb[:],
                    in_=pool_ps[:],
                    func=Act.Copy,
                    scale=rec_t[:, g : g + 1],
                )

                pt_ps = ps_pt.tile([128, D], bf16)
                for c in range(4):
                    nc.tensor.transpose(
                        pt_ps[:, c * 128 : (c + 1) * 128],
                        pooled_sb[:, c * 128 : (c + 1) * 128],
                        id_t[:],
                    )
                pt_sb = ptp.tile([128, D], bf16)
                nc.scalar.copy(pt_sb[:], pt_ps[:])

                o_ps = ps_o.tile([128, D], fp32)
                nc.tensor.matmul(
                    out=o_ps[:], lhsT=ones_t[:], rhs=b_t[:], start=True, stop=False
                )
                for c in range(4):
                    nc.tensor.matmul(
                        out=o_ps[:],
                        lhsT=pt_sb[:, c * 128 : (c + 1) * 128],
                        rhs=wt_t[:, c * D : (c + 1) * D],
                        start=False,
                        stop=(c == 3),
                    )
                nc.scalar.copy(out_sb[:, g * D : (g + 1) * D], o_ps[:])
            o0 = start_row // RATIO
            nc.sync.dma_start(
                out=out[o0 : o0 + (J * 128) // RATIO, :].rearrange(
                    "(p j) d -> p (j d)", j=G
                ),
                in_=out_sb[:],
            )

        rep_loop = tc.For_i(0, reps, 1) if reps > 1 else contextlib.nullcontext()
        with rep_loop:
            states = {}
            PREFETCH = 2
            load_consts()
            staged = {}
            for t in range(n_tiles):
                start, nrows, is_staged = tiles[t]
                if is_staged:
                    x_t, stage = stage_dma(start, nrows)
                    staged[t] = stage
                    states[t] = {"x": x_t, "J": nrows // 128, "start": start}
            # prologue SWDGE: first two non-staged tiles
            for t in range(N_STAGED, N_STAGED + PREFETCH):
                start, nrows, _ = tiles[t]
                states[t] = {
                    "x": load_tile(start, nrows),
                    "J": nrows // 128,
                    "start": start,
                }
            for i in range(n_tiles + 2):
                nxt = i + PREFETCH
                if N_STAGED + PREFETCH <= nxt < n_tiles:
                    start, nrows, _ = tiles[nxt]
                    states[nxt] = {
                        "x": load_tile(start, nrows),
                        "J": nrows // 128,
                        "start": start,
                    }
                if 1 <= i <= n_tiles:
                    exp_stage(states[i - 1])
                    softmax_finish(states[i - 1])
                    d_stage(states[i - 1])
                if i < n_tiles:
                    if i in staged:
                        cast_stage(states[i]["x"], staged.pop(i))
                    scores_front(states[i])
                if i >= 2:
                    pe_block(states[i - 2])
                    del states[i - 2]

    nc.compile()
    return nc


def get_nc(rows_per_core=ROWS_PER_CORE, reps=1):
    key = (rows_per_core, reps)
    if key not in _NC_CACHE:
        _NC_CACHE[key] = _build_nc(rows_per_core, reps)
    return _NC_CACHE[key]


def _aux_inputs(query, w, b):
    import ml_dtypes

    bf16 = ml_dtypes.bfloat16
    q = np.asarray(query, dtype=np.float32)
    qbc = np.ascontiguousarray(np.broadcast_to(q.astype(bf16), (128, D)))
    wtb = np.ascontiguousarray(np.asarray(w, dtype=np.float32).T.astype(bf16))
    ident = np.eye(128, dtype=bf16)
    ones1 = np.ones((1, 128), dtype=bf16)
    brow = np.asarray(b, dtype=np.float32).astype(bf16).reshape(1, D)
    return {
        "qbc": qbc,
        "wtb": wtb,
        "ident": ident,
        "ones1": ones1,
        "brow": brow,
    }


def make_in_maps(chunk, query, w, b, rows_per_core=ROWS_PER_CORE, n_cores=N_CORES):
    chunk = np.asarray(chunk, dtype=np.float32)
    aux = _aux_inputs(query, w, b)
    return [
        {
            "chunk": np.ascontiguousarray(
                chunk[c * rows_per_core : (c + 1) * rows_per_core]
            ),
            **aux,
        }
        for c in range(n_cores)
    ]


def kernel(chunk, query, w, b, trace=False):
    from concourse.bass_utils import run_bass_kernel_spmd

    nc = get_nc(ROWS_PER_CORE)
    in_maps = make_in_maps(chunk, query, w, b)
    res = run_bass_kernel_spmd(nc, in_maps, list(range(N_CORES)), trace=trace)
    out = np.concatenate(
        [np.asarray(res.results[c]["out"]) for c in range(N_CORES)], axis=0
    ).astype(np.float32)
    kernel.last_results = res
    return out


# revision 10
# speedup vs baseline: 1.1729x; 1.1729x over previous
"""Trainium2 Bass kernel for AttentionPoolCompressor.

Computation (matches the reference nn.Module):
    x = chunk.reshape(N, 4, 512)
    scores = einsum('d,nrd->nr', query, x) / sqrt(512)
    attn   = softmax(scores, axis=-1)
    pooled = einsum('nr,nrd->nd', attn, x)
    out    = pooled @ w.T + b

Sharding: chunk rows are split contiguously across 8 NeuronCores (each pools
its own L/8 rows independently); query / w / b are replicated.  No
collectives; each core writes its own slice of the output.

Design (v5).  HBM floor: 64MiB fp32 in + 8MiB bf16 out + consts ~= 76.7MB
@358GB/s = 214us; the kernel aims to keep the DMA stream gap-free and the
compute cadence strictly below the 13.2us/tile DMA cadence.
  * Variable tile schedule: 4x512-row staged sub-tiles (HWDGE fp32 +
    DVE cast; compute starts ~8us), 14x2048-row SWDGE-cast main tiles,
    4x512-row SWDGE tail sub-tiles (shrinks the post-last-input drain
    chain from ~22us to ~8us).
  * Consts (q, id, wt, ones, b) are FIRST in the HWDGE FIFO — in v4 they
    sat behind 8.4MB of stage loads and gated the first diag/proj at 55us.
  * xp bufs=6: with 5, tile k+5's SWDGE load waited on PE(k) retiring its
    x tile, starving the input stream for ~20us mid-kernel.
  * Output is bf16 (halves output HBM bytes; host upcasts — measured
    rel err 4.9e-3 vs the 2e-2 gate).
  * Scores: bf16 2x-mode pair-add tree (D 512->256->128->64) + segmented
    tensor_reduce.  Fused accum ops (tensor_tensor_reduce ISA op crashes
    the exec unit; scalar_tensor_tensor / custom affine_mul_reduce run
    1x-mode ~0.77us/j) all lose to the tree (~9.7us/tile).
  * Engine split per 2048-row tile (measured: DVE TT bf16 0.55ns/e 2x,
    reduce 1.08, ACT 0.54ns/e + 410ns overhead, PE ~0.56ns/beat):
      DVE ~12.0us: cast (staged only), score tree, group sums+recip,
        12/16 of the diag build (broadcast TT, 1x).
      ACT ~11.7us: exp (bf16 out), 4/16 diag, pooled copy (folds 1/sum
        as per-partition scale), pooledT copy, out copy.
      PE  ~10.7us: pool (diag matmuls), transposes, proj, bias matmul.
  * Two-tile-lag software pipeline as before (scores i | softmax i-1 |
    PE i-2) so no in-order engine queue head-blocks.
"""

import math
import sys

import numpy as np

if "/opt/trn_rl_repo" not in sys.path:
    sys.path.insert(0, "/opt/trn_rl_repo")

D = 512
RATIO = 4
N_CORES = 8
L_FULL = 262144
ROWS_PER_CORE = L_FULL // N_CORES  # 32768

MAIN_ROWS = 2048  # main tile size
SUB_ROWS = 512  # staged/tail sub-tile size
N_STAGED = 4  # leading sub-tiles via HWDGE fp32 + DVE cast
N_TAIL = 4  # trailing sub-tiles (short drain chain)

OUT_BF16 = True

_NC_CACHE = {}


def _tile_list(rows):
    """[(row_start, nrows, staged)] — staged prefix, main body, small tail."""
    tiles = [(i * SUB_ROWS, SUB_ROWS, True) for i in range(N_STAGED)]
    r = N_STAGED * SUB_ROWS
    tail = N_TAIL * SUB_ROWS
    while rows - r - tail >= MAIN_ROWS:
        tiles.append((r, MAIN_ROWS, False))
        r += MAIN_ROWS
    while r < rows:
        tiles.append((r, SUB_ROWS, False))
        r += SUB_ROWS
    assert r == rows
    return tiles


def _build_nc(rows_per_core, reps=1):
    import contextlib
    from contextlib import ExitStack

    import concourse.bacc as bacc
    import concourse.tile as tile
    from concourse import mybir

    fp32 = mybir.dt.float32
    bf16 = mybir.dt.bfloat16
    Alu = mybir.AluOpType
    Act = mybir.ActivationFunctionType
    X = mybir.AxisListType.X

    tiles = _tile_list(rows_per_core)
    n_tiles = len(tiles)
    out_rows = rows_per_core // RATIO
    inv_sqrt_d = 1.0 / math.sqrt(D)

    nc = bacc.Bacc("TRN2", target_bir_lowering=False, debug=False)
    chunk = nc.dram_tensor("chunk", [rows_per_core, D], fp32, kind="ExternalInput").ap()
    wtb = nc.dram_tensor("wtb", [D, D], bf16, kind="ExternalInput").ap()
    qbc = nc.dram_tensor("qbc", [128, D], bf16, kind="ExternalInput").ap()
    ident = nc.dram_tensor("ident", [128, 128], bf16, kind="ExternalInput").ap()
    ones1 = nc.dram_tensor("ones1", [1, 128], bf16, kind="ExternalInput").ap()
    brow = nc.dram_tensor("brow", [1, D], bf16, kind="ExternalInput").ap()
    out_dt = bf16 if OUT_BF16 else fp32
    out = nc.dram_tensor("out", [out_rows, D], out_dt, kind="ExternalOutput").ap()

    JMAX = MAIN_ROWS // 128  # 16

    with tile.TileContext(nc) as tc, ExitStack() as ctx:
        const = ctx.enter_context(tc.tile_pool(name="const", bufs=1))
        xp = ctx.enter_context(tc.tile_pool(name="xp", bufs=6))
        xps = ctx.enter_context(tc.tile_pool(name="xps", bufs=5))
        prp = ctx.enter_context(tc.tile_pool(name="prp", bufs=1))
        l1p = ctx.enter_context(tc.tile_pool(name="l1p", bufs=1))
        l2p = ctx.enter_context(tc.tile_pool(name="l2p", bufs=1))
        l3p = ctx.enter_context(tc.tile_pool(name="l3p", bufs=1))
        s_p = ctx.enter_context(tc.tile_pool(name="s_p", bufs=3))
        e_p = ctx.enter_context(tc.tile_pool(name="e_p", bufs=3))
        gs_p = ctx.enter_context(tc.tile_pool(name="gs_p", bufs=3))
        rec_p = ctx.enter_context(tc.tile_pool(name="rec_p", bufs=3))
        dp = ctx.enter_context(tc.tile_pool(name="dp", bufs=3))
        pooledp = ctx.enter_context(tc.tile_pool(name="pooledp", bufs=2))
        ptp = ctx.enter_context(tc.tile_pool(name="ptp", bufs=2))
        outp = ctx.enter_context(tc.tile_pool(name="outp", bufs=3))
        fencep = ctx.enter_context(tc.tile_pool(name="fencep", bufs=1))
        stagep = ctx.enter_context(tc.tile_pool(name="stagep", bufs=3))
        ps_pool = ctx.enter_context(tc.tile_pool(name="ps_pool", bufs=2, space="PSUM"))
        ps_pt = ctx.enter_context(tc.tile_pool(name="ps_pt", bufs=2, space="PSUM"))
        ps_o = ctx.enter_context(tc.tile_pool(name="ps_o", bufs=4, space="PSUM"))

        wt_t = const.tile([128, 4 * D], bf16)
        q_t = const.tile([128, D], bf16)
        id_t = const.tile([128, 128], bf16)
        ones_t = const.tile([1, 128], bf16)
        b_t = const.tile([1, D], bf16)

        def load_consts():
            # consts go FIRST in the HWDGE FIFO: q for scores, id for
            # diag/transposes, wt/ones/b for the proj — all needed within
            # the first ~10us.
            nc.sync.dma_start(out=q_t[:], in_=qbc[:, :])
            nc.sync.dma_start(out=id_t[:], in_=ident[:, :])
            nc.sync.dma_start(out=ones_t[:], in_=ones1[:, :])
            nc.sync.dma_start(out=b_t[:], in_=brow[:, :])
            for c in range(4):
                nc.sync.dma_start(
                    out=wt_t[:, c * D : (c + 1) * D],
                    in_=wtb[c * 128 : (c + 1) * 128, :],
                )

        def src_ap(start, nrows):
            j = nrows // 128
            return chunk[start : start + nrows, :].rearrange("(p j) d -> p (j d)", j=j)

        def load_tile(start, nrows):
            j = nrows // 128
            pool = xp if nrows == MAIN_ROWS else xps
            x_t = pool.tile([128, j * D], bf16)
            nc.gpsimd.dma_start(out=x_t[:], in_=src_ap(start, nrows))
            return x_t

        def stage_dma(start, nrows):
            j = nrows // 128
            x_t = xps.tile([128, j * D], bf16)
            stage = stagep.tile([128, j * D], fp32)
            nc.sync.dma_start(out=stage[:], in_=src_ap(start, nrows))
            return x_t, stage

        def cast_stage(x_t, stage):
            nc.vector.tensor_copy(x_t[:], stage[:])

        def scores_front(st):
            """DVE: 2x-mode pair-add tree + segmented reduce -> s [128,J]."""
            x_t, J = st["x"], st["J"]
            pv = x_t[:].rearrange("p (j d) -> p j d", j=J)
            pr = prp.tile([128, J * D], bf16)
            prv = pr[:].rearrange("p (j d) -> p j d", j=J)
            nc.vector.tensor_tensor(
                prv, pv, q_t[:].unsqueeze(1).broadcast_to((128, J, D)), Alu.mult
            )
            l1 = l1p.tile([128, J * 256], bf16)
            v1 = l1[:].rearrange("p (j d) -> p j d", j=J)
            nc.vector.tensor_tensor(v1, prv[:, :, 0:256], prv[:, :, 256:512], Alu.add)
            l2 = l2p.tile([128, J * 128], bf16)
            v2 = l2[:].rearrange("p (j d) -> p j d", j=J)
            nc.vector.tensor_tensor(v2, v1[:, :, 0:128], v1[:, :, 128:256], Alu.add)
            l3 = l3p.tile([128, J * 64], bf16)
            v3 = l3[:].rearrange("p (j d) -> p j d", j=J)
            nc.vector.tensor_tensor(v3, v2[:, :, 0:64], v2[:, :, 64:128], Alu.add)
            s_t = s_p.tile([128, J], fp32)
            nc.vector.tensor_reduce(s_t[:], v3, axis=X, op=Alu.add)
            st["s"] = s_t

        def exp_stage(st):
            J = st["J"]
            e_t = e_p.tile([128, J], fp32)  # ACT diag scale AP must be fp32
            nc.scalar.activation(
                out=e_t[:], in_=st["s"][:], func=Act.Exp, scale=inv_sqrt_d
            )
            st["e"] = e_t

        def softmax_finish(st):
            J, G = st["J"], st["J"] // RATIO
            gs_t = gs_p.tile([128, G], fp32)
            nc.vector.tensor_reduce(
                gs_t[:],
                st["e"][:].rearrange("p (g r) -> p g r", g=G),
                axis=X,
                op=Alu.add,
            )
            rec_t = rec_p.tile([128, G], fp32)
            nc.vector.reciprocal(rec_t[:], gs_t[:])
            st["rec"] = rec_t

        def d_stage(st):
            """UNNORMALIZED diag(e) blocks; 3/4 on DVE, 1/4 on ACT."""
            J = st["J"]
            dve_j = (3 * J) // 4
            d_t = dp.tile([128, J * 128], bf16)
            nc.vector.tensor_tensor(
                d_t[:, : dve_j * 128].rearrange("p (j m) -> p j m", j=dve_j),
                id_t[:].unsqueeze(1).broadcast_to((128, dve_j, 128)),
                st["e"][:, :dve_j].unsqueeze(2).broadcast_to((128, dve_j, 128)),
                Alu.mult,
            )
            for j in range(dve_j, J):
                nc.scalar.activation(
                    out=d_t[:, j * 128 : (j + 1) * 128],
                    in_=id_t[:],
                    func=Act.Copy,
                    scale=st["e"][:, j : j + 1],
                )
            st["d"] = d_t

        def pe_block(st):
            x_t, d_t, rec_t = st["x"], st["d"], st["rec"]
            start_row, J = st["start"], st["J"]
            G = J // RATIO
            out_sb = outp.tile([128, G * D], bf16)
            for g in range(G):
                pool_ps = ps_pool.tile([128, D], fp32)
                for r in range(RATIO):
                    j = g * RATIO + r
                    nc.tensor.matmul(
                        out=pool_ps[:],
                        lhsT=d_t[:, j * 128 : (j + 1) * 128],
                        rhs=x_t[:, j * D : (j + 1) * D],
                        start=(r == 0),
                        stop=(r == RATIO - 1),
                    )
                pooled_sb = pooledp.tile([128, D], bf16)
                nc.scalar.activation(
                    out=pooled_sb[:],
                    in_=pool_ps[:],
                    func=Act.Copy,
                    scale=rec_t[:, g : g + 1],
                )

                pt_ps = ps_pt.tile([128, D], bf16)
                for c in range(4):
                    nc.tensor.transpose(
                        pt_ps[:, c * 128 : (c + 1) * 128],
                        pooled_sb[:, c * 128 : (c + 1) * 128],
                        id_t[:],
                    )
                pt_sb = ptp.tile([128, D], bf16)
                nc.scalar.copy(pt_sb[:], pt_ps[:])

                o_ps = ps_o.tile([128, D], fp32)
                nc.tensor.matmul(
                    out=o_ps[:], lhsT=ones_t[:], rhs=b_t[:], start=True, stop=False
                )
                for c in range(4):
                    nc.tensor.matmul(
                        out=o_ps[:],
                        lhsT=pt_sb[:, c * 128 : (c + 1) * 128],
                        rhs=wt_t[:, c * D : (c + 1) * D],
                        start=False,
                        stop=(c == 3),
                    )
                nc.scalar.copy(out_sb[:, g * D : (g + 1) * D], o_ps[:])
            o0 = start_row // RATIO
            nc.sync.dma_start(
                out=out[o0 : o0 + (J * 128) // RATIO, :].rearrange(
                    "(p j) d -> p (j d)", j=G
                ),
                in_=out_sb[:],
            )

        rep_loop = tc.For_i(0, reps, 1) if reps > 1 else contextlib.nullcontext()
        with rep_loop:
            states = {}
            PREFETCH = 2
            load_consts()
            staged = {}
            for t in range(n_tiles):
                start, nrows, is_staged = tiles[t]
                if is_staged:
                    x_t, stage = stage_dma(start, nrows)
                    staged[t] = stage
                    states[t] = {"x": x_t, "J": nrows // 128, "start": start}
            # SWDGE fence: the gpsimd engine (which emits all SWDGE
            # descriptors) blocks until the last staged quarter's data is
            # in SBUF.  Without this the deep SWDGE queue starves the
            # HWDGE stage loads (measured 12 GB/s for 60us) and the
            # compute pipeline start slips to ~30us.
            fence_t = fencep.tile([128, 1], fp32)
            nc.gpsimd.tensor_copy(fence_t[:], staged[N_STAGED - 1][:, 0:1])
            # prologue SWDGE: first two non-staged tiles
            for t in range(N_STAGED, N_STAGED + PREFETCH):
                start, nrows, _ = tiles[t]
                states[t] = {
                    "x": load_tile(start, nrows),
                    "J": nrows // 128,
                    "start": start,
                }
            for i in range(n_tiles + 2):
                nxt = i + PREFETCH
                if N_STAGED + PREFETCH <= nxt < n_tiles:
                    start, nrows, _ = tiles[nxt]
                    states[nxt] = {
                        "x": load_tile(start, nrows),
                        "J": nrows // 128,
                        "start": start,
                    }
                if 1 <= i <= n_tiles:
                    exp_stage(states[i - 1])
                    softmax_finish(states[i - 1])
                    d_stage(states[i - 1])
                if i < n_tiles:
                    if i in staged:
                        cast_stage(states[i]["x"], staged.pop(i))
                    scores_front(states[i])
                if i >= 2:
                    pe_block(states[i - 2])
                    del states[i - 2]

    nc.compile()
    return nc


def get_nc(rows_per_core=ROWS_PER_CORE, reps=1):
    key = (rows_per_core, reps)
    if key not in _NC_CACHE:
        _NC_CACHE[key] = _build_nc(rows_per_core, reps)
    return _NC_CACHE[key]


def _aux_inputs(query, w, b):
    import ml_dtypes

    bf16 = ml_dtypes.bfloat16
    q = np.asarray(query, dtype=np.float32)
    qbc = np.ascontiguousarray(np.broadcast_to(q.astype(bf16), (128, D)))
    wtb = np.ascontiguousarray(np.asarray(w, dtype=np.float32).T.astype(bf16))
    ident = np.eye(128, dtype=bf16)
    ones1 = np.ones((1, 128), dtype=bf16)
    brow = np.asarray(b, dtype=np.float32).astype(bf16).reshape(1, D)
    return {
        "qbc": qbc,
        "wtb": wtb,
        "ident": ident,
        "ones1": ones1,
        "brow": brow,
    }


def make_in_maps(chunk, query, w, b, rows_per_core=ROWS_PER_CORE, n_cores=N_CORES):
    chunk = np.asarray(chunk, dtype=np.float32)
    aux = _aux_inputs(query, w, b)
    return [
        {
            "chunk": np.ascontiguousarray(
                chunk[c * rows_per_core : (c + 1) * rows_per_core]
            ),
            **aux,
        }
        for c in range(n_cores)
    ]


def kernel(chunk, query, w, b, trace=False):
    from concourse.bass_utils import run_bass_kernel_spmd

    nc = get_nc(ROWS_PER_CORE)
    in_maps = make_in_maps(chunk, query, w, b)
    res = run_bass_kernel_spmd(nc, in_maps, list(range(N_CORES)), trace=trace)
    out = np.concatenate(
        [np.asarray(res.results[c]["out"]) for c in range(N_CORES)], axis=0
    ).astype(np.float32)
    kernel.last_results = res
    return out
